# revision 5
# baseline (speedup 1.0000x reference)
"""Cross-attention kernel v11 for Trainium2, data-parallel over batch across 8 cores.

v11 vs v10: normalize multiplies read the PV accumulator directly from PSUM
(the 4-slot accumulator pool absorbs the longer hold), dropping the two
per-head evacuation copies from DVE; freed SBUF funds 2 extra est slots.

v8 vs v7 (HW-microbenchmark-driven):
  - PV matmul weights padded to 128 columns (V tile [P,SC,H,128]): LDWEIGHTS
    with NumWeights==128 triggers Fast Weight Load; 97-col weights measured
    677 ns/MM vs 179 ns/MM at 128 cols. Saves ~60 us of PE time.
  - Softmax denominator reciprocal via reciprocal_approx_fast (custom DVE op,
    ~0.6 us vs 3.2 us for the iterative [1,512] reciprocal). Broadcast in f32.
  - Phase-1 Q/K PSUM->SBUF copies alternate DVE/ACT (ACT idle in phase 1).
  - Output DMA: one [128,768] f32 transfer per q-block on the ACT HWDGE ring
    (SP ring is busy with input loads; ACT is idle in the tail).
"""

import sys

if "/opt/trn_rl_repo" not in sys.path:
    sys.path.insert(0, "/opt/trn_rl_repo")

import ml_dtypes
import numpy as np

import concourse.bass as bass
import concourse.mybir as mybir
from concourse import bacc
from concourse import library_config
import concourse.tile as tile
from concourse.bass_utils import run_bass_kernel_spmd

F32 = mybir.dt.float32
BF16 = mybir.dt.bfloat16
F8 = mybir.dt.float8e4

N, S, D = 8, 1024, 768
H, HD = 8, 96
P = 128
C = D // P
SC = S // P
W_SCALE = 16.0
SCALE = 1.0 / (float(np.sqrt(D)) * W_SCALE * W_SCALE)
N_CORES = 8
EST_BUFS = 26


def build_program(reps=1, serialize_reps=False):
    nc = bacc.Bacc(None, target_bir_lowering=False)

    qT8 = nc.dram_tensor("qT8", [D, S], F8, kind="ExternalInput")
    kvT8 = nc.dram_tensor("kvT8", [D, S], F8, kind="ExternalInput")
    wqT8 = nc.dram_tensor("wqT8", [D, D], F8, kind="ExternalInput")
    wkT8 = nc.dram_tensor("wkT8", [D, D], F8, kind="ExternalInput")
    kvT = nc.dram_tensor("kvT", [D, S], BF16, kind="ExternalInput")
    wvT = nc.dram_tensor("wvT", [D, D], BF16, kind="ExternalInput")
    wpT = nc.dram_tensor("wpT", [D, D], BF16, kind="ExternalInput")
    bias = nc.dram_tensor("bias", [1, D], F32, kind="ExternalInput")
    out = nc.dram_tensor("out", [S, D], F32, kind="ExternalOutput")

    with tile.TileContext(nc) as tc:
        persist = tc.alloc_tile_pool(name="persist", bufs=1)
        QT = persist.tile([HD, H, S], BF16, tag="QT")
        KT = persist.tile([HD, H, S], BF16, tag="KT")
        V = persist.tile([P, SC, H, P], BF16, tag="V")
        outhT = persist.tile([HD, H, S], BF16, tag="outhT")
        qa8 = persist.tile([P, C, S], F8, tag="qa8")
        kva8 = persist.tile([P, C, S], F8, tag="kva8")
        wq8 = persist.tile([P, C, D], F8, tag="wq8")
        wk8 = persist.tile([P, C, D], F8, tag="wk8")
        kva = persist.tile([P, C, S], BF16, tag="kva")
        wv_t = persist.tile([P, C, D], BF16, tag="wv")
        wp_t = persist.tile([HD, H, D], BF16, tag="wp")
        bias_t = persist.tile([1, D], F32, tag="bias")
        bias_bc = persist.tile([P, D], F32, tag="bias_bc")
        pwsrc = persist.tile([P, 512], BF16, tag="pwsrc")
        stgA = persist.tile([64, 512], BF16, tag="stgA")
        stgB = persist.tile([64, 512], BF16, tag="stgB")
        rstp = [
            [
                persist.tile([32, 512], BF16, tag=f"rst{a}{q}", name=f"rst{a}{q}")
                for q in range(2)
            ]
            for a in range(2)
        ]

        with (
            tc.tile_pool(name="expst", bufs=EST_BUFS) as estpool,
            tc.tile_pool(name="smmisc", bufs=2) as mpool,
            tc.tile_pool(name="osb", bufs=3) as opool,
            tc.tile_pool(name="stpsum", bufs=2, space="PSUM") as stpsum,
            tc.tile_pool(name="accpsum", bufs=4, space="PSUM") as accpsum,
        ):
            for _rep in range(reps):
                if serialize_reps and _rep > 0:
                    tc.strict_bb_all_engine_barrier()
                nc.gpsimd.load_library(library_config.attn)
                # Pre-warm the Exp activation table during the DMA lead-in.
                warm = mpool.tile([1, 8], F32, tag="warm")
                warm_o = mpool.tile([1, 8], BF16, tag="warm_o")
                nc.vector.memset(warm[:], 0.0)
                nc.scalar.activation(
                    warm_o[:], warm[:], mybir.ActivationFunctionType.Exp, scale=1.0
                )
                if _rep == 0:
                    nc.vector.memset(pwsrc[:], 0.0)
                    nc.vector.memset(stgA[:], 1.0)
                    nc.vector.memset(stgB[:], 1.0)
                    for a in range(2):
                        for q in range(2):
                            nc.vector.memset(rstp[a][q][:], 1.0)
                # PE pre-warm: ~4us of dummy matmuls during the DMA lead-in so
                # the HAM clock-gate opens before the first real projection.
                for pw in range(8):
                    pwt = stpsum.tile([P, 2, 512], F32, tag="st", name=f"pw{pw}")
                    nc.tensor.matmul(
                        pwt[:, 0], pwsrc[:, 0:P], pwsrc[:], start=True, stop=True
                    )
                # Q/K fp8 path first: these gate the first matmuls. One
                # rearranged DMA each (the first projection chain needs the
                # whole tensor anyway; fewer DGE descriptor-gens frees the SP
                # ring sooner).
                nc.sync.dma_start(qa8[:], qT8.rearrange("(c p) s -> p c s", p=P))
                nc.sync.dma_start(wq8[:], wqT8.rearrange("(c p) o -> p c o", p=P))
                nc.sync.dma_start(kva8[:], kvT8.rearrange("(c p) s -> p c s", p=P))
                nc.sync.dma_start(wk8[:], wkT8.rearrange("(c p) o -> p c o", p=P))
                nc.sync.dma_start(kva[:], kvT.rearrange("(c p) s -> p c s", p=P))
                nc.sync.dma_start(wv_t[:], wvT.rearrange("(c p) o -> p c o", p=P))
                nc.sync.dma_start(wp_t[:], wpT.rearrange("(h d) o -> d h o", d=HD))
                nc.sync.dma_start(bias_t[:], bias[:, :])
                if _rep == 0:
                    # cols 96..127 of each V slice: ones (denominator row) then
                    # zeros so the junk PSUM rows 97..127 stay finite.
                    nc.vector.memset(V[:, :, :, HD:], 0.0)
                nc.vector.memset(V[:, :, :, HD], 1.0)
                nc.gpsimd.partition_broadcast(bias_bc[:], bias_t[:], channels=P)

                # ====== Phase 1: Q/K projections (fp8 DoubleRow) ======
                ci = 0
                for h in range(H):
                    for w8, acts, dstT in ((wq8, qa8, QT), (wk8, kva8, KT)):
                        for sh in range(2):
                            ps = accpsum.tile([P, 512], F32, tag="acc")
                            for cp in range(C // 2):
                                nc.tensor.matmul(
                                    ps[0:HD],
                                    w8[:, 2 * cp:2 * cp + 2, h * HD:(h + 1) * HD],
                                    acts[:, 2 * cp:2 * cp + 2, sh * 512:(sh + 1) * 512],
                                    start=(cp == 0),
                                    stop=(cp == C // 2 - 1),
                                    perf_mode=mybir.MatmulPerfMode.DoubleRow,
                                )
                            # ACT evacuates only head 0 (keeps its FIFO free
                            # so the exp stream starts ~5us in); DVE handles
                            # the rest, trickling into early phase 2.
                            if h == 0:
                                nc.scalar.copy(
                                    dstT[:, h, sh * 512:(sh + 1) * 512], ps[0:HD]
                                )
                            else:
                                nc.vector.tensor_copy(
                                    dstT[:, h, sh * 512:(sh + 1) * 512], ps[0:HD]
                                )
                            ci += 1

                # ====== Phase 2: slot-scheduled attention ======
                est_tiles = {}

                def emit_scores(h, kc):
                    st = stpsum.tile([P, 2, 512], F32, tag="st")
                    for qh in range(2):
                        nc.tensor.matmul(
                            st[:, qh],
                            KT[:, h, kc * P:(kc + 1) * P],
                            QT[:, h, qh * 512:(qh + 1) * 512],
                            start=True,
                            stop=True,
                        )
                    est = estpool.tile([P, 2, 512], BF16, tag="est")
                    nc.scalar.activation(
                        est[:], st[:], mybir.ActivationFunctionType.Exp, scale=SCALE
                    )
                    est_tiles[(h, kc)] = est

                # fp8 DoubleRow V projection: lhsT=kva8 chunk (the "weight"),
                # rhs=wv8 -> ps[s-block, dh*384..]; unscale 1/W_SCALE on copy.
                def emit_vproj(sc, dh):
                    psw = accpsum.tile([P, 512], F32, tag="acc", name=f"vp{sc}_{dh}")
                    ps = psw[:, 0:384]
                    for c in range(C):
                        nc.tensor.matmul(
                            ps[:],
                            kva[:, c, sc * P:(sc + 1) * P],
                            wv_t[:, c, dh * 384:(dh + 1) * 384],
                            start=(c == 0),
                            stop=(c == C - 1),
                        )
                    nc.vector.tensor_copy(
                        V[:, sc, dh * 4:(dh + 1) * 4, 0:HD],
                        ps[:].rearrange("p (h d) -> p h d", d=HD),
                    )

                # PV is emitted as per-slot matmul pairs; chain state per head.
                pv_state = {}

                def start_pv(h):
                    pv_state[h] = {"i": 0, "po": {}}

                def finish_norm(h, stt):
                    # double-transpose reciprocal: both qh denominator rows
                    # land strided over 32 partitions, reciprocal runs 32-wide
                    # (0.3us) instead of twice 3.2us single-lane.
                    stg = stt["stg"]
                    stgT = mpool.tile([64, 512], BF16, tag="stgT", name=f"sT{h}")
                    nc.vector.transpose(stgT[:], stg[:])
                    for qh in range(2):
                        # den row qh sits at stgT[32qh..32qh+31, 32j] after the
                        # block transpose; strided recip writes lane position 0
                        # so the back-transpose homes the row at partition 0
                        # (AP starts must be 0/32/64/96).
                        rst = rstp[h % 2][qh]
                        rin = stgT[32 * qh:32 * qh + 32].rearrange(
                            "p (j i) -> p j i", i=32
                        )[:, :, 0:1]
                        rout = rst[:].rearrange("p (j i) -> p j i", i=32)[
                            :, :, 0:1
                        ]
                        with nc.allow_low_precision(
                            reason="recip feeds bf16 outh scaling"
                        ):
                            nc.vector.reciprocal(rout, rin)
                        rT = mpool.tile(
                            [32, 512], BF16, tag=f"rT{qh}", name=f"rT{h}_{qh}"
                        )
                        nc.vector.transpose(rT[:], rst[:])
                        bc = mpool.tile(
                            [HD, 512], BF16, tag="bc", name=f"bc_{h}_{qh}"
                        )
                        nc.gpsimd.partition_broadcast(
                            bc[:], rT[0:1], channels=HD
                        )
                        nc.vector.tensor_mul(
                            outhT[:, h, qh * 512:(qh + 1) * 512],
                            stt[f"po{qh}"][0:HD], bc[:],
                        )

                def emit_pv_mms(h, count):
                    stt = pv_state[h]
                    for _ in range(count):
                        i = stt["i"]
                        if i >= 16:
                            return
                        qh, kc = divmod(i, SC)
                        if kc == 0:
                            stt["po"][qh] = accpsum.tile(
                                [P, 512], F32, tag="acc", name=f"po_{h}_{qh}"
                            )
                        po = stt["po"][qh]
                        nc.tensor.matmul(
                            po[:],
                            V[:, kc, h, :],
                            est_tiles[(h, kc)][:, qh],
                            start=(kc == 0),
                            stop=(kc == SC - 1),
                        )
                        stt["i"] = i + 1
                        if kc == SC - 1:
                            # copy out the denominator row only; po stays in
                            # PSUM until the normalize multiply (the 4-slot
                            # accumulator pool covers the ~3us hold).
                            if qh == 0:
                                stt["stg"] = stgA if h % 2 == 0 else stgB
                            nc.vector.tensor_copy(
                                stt["stg"][32 * qh:32 * qh + 1], po[HD:HD + 1]
                            )
                            stt[f"po{qh}"] = po
                            if qh == 1:
                                finish_norm(h, stt)

                # V jobs dh-major: first 8 produce the V halves PV(h0..3) needs.
                vjobs = [(sc, dh) for dh in range(2) for sc in range(SC)]
                vj = 0
                for h in range(H):
                    for kc in range(SC):
                        emit_scores(h, kc)
                        if h < 4 and kc % 2 == 0 and vj < len(vjobs):
                            emit_vproj(*vjobs[vj])
                            vj += 1
                        if 2 <= h <= 5:
                            hh = h - 2
                            if hh not in pv_state:
                                start_pv(hh)
                            emit_pv_mms(hh, 2)
                        elif h == 6:
                            if kc < 4:
                                if 4 not in pv_state:
                                    start_pv(4)
                                emit_pv_mms(4, 4)
                            else:
                                if 5 not in pv_state:
                                    start_pv(5)
                                emit_pv_mms(5, 4)
                        elif h == 7:
                            if kc < 4:
                                if 6 not in pv_state:
                                    start_pv(6)
                                emit_pv_mms(6, 4)
                            if kc >= 1:
                                if 7 not in pv_state:
                                    start_pv(7)
                                emit_pv_mms(7, 1)
                    if 2 <= h <= 5:
                        emit_pv_mms(h - 2, 16)
                    elif h == 6:
                        emit_pv_mms(4, 16)
                        emit_pv_mms(5, 16)
                emit_pv_mms(6, 16)
                if 7 not in pv_state:
                    start_pv(7)
                emit_pv_mms(7, 16)

                # ====== Phase 3: output projection ======
                for qc in range(SC):
                    ot = opool.tile([P, D], F32, tag="ot")
                    for oh in range(2):
                        psw = accpsum.tile([P, 512], F32, tag="acc",
                                           name=f"op{qc}_{oh}")
                        ps = psw[:, 0:384]
                        for h in range(H):
                            nc.tensor.matmul(
                                ps[:],
                                outhT[:, h, qc * P:(qc + 1) * P],
                                wp_t[:, h, oh * 384:(oh + 1) * 384],
                                start=(h == 0),
                                stop=(h == H - 1),
                            )
                        nc.vector.tensor_add(
                            ot[:, oh * 384:(oh + 1) * 384], ps[:],
                            bias_bc[:, oh * 384:(oh + 1) * 384],
                        )
                    # One [128,768] f32 store per q-block on the ACT HWDGE
                    # ring (idle in the tail; SP ring carries the input loads).
                    nc.scalar.dma_start(out[qc * P:(qc + 1) * P, :], ot[:])

        persist.release()

    nc.compile()
    return nc


_NC_CACHE = {}


def _get_nc(reps=1, serialize_reps=False):
    key = (reps, serialize_reps)
    if key not in _NC_CACHE:
        _NC_CACHE[key] = build_program(reps, serialize_reps)
    return _NC_CACHE[key]


def _bf16(x):
    return np.ascontiguousarray(np.asarray(x, np.float32).astype(ml_dtypes.bfloat16))


def _f8(x):
    return np.ascontiguousarray(np.asarray(x, np.float32).astype(ml_dtypes.float8_e4m3))


def make_in_maps(q, kv, wq, wk, wv, w_proj, b_proj):
    q = np.asarray(q, np.float32)
    kv = np.asarray(kv, np.float32)
    qT8 = _f8(q.transpose(0, 2, 1))
    kvT8 = _f8(kv.transpose(0, 2, 1))
    wqT8 = _f8(np.asarray(wq, np.float32).T * W_SCALE)
    wkT8 = _f8(np.asarray(wk, np.float32).T * W_SCALE)
    kvT = _bf16(kv.transpose(0, 2, 1))
    wvT = _bf16(np.asarray(wv, np.float32).T)
    wpT = _bf16(np.asarray(w_proj, np.float32).T)
    b2d = np.ascontiguousarray(np.asarray(b_proj, np.float32).reshape(1, D))
    return [
        {
            "qT8": qT8[i], "kvT8": kvT8[i], "kvT": kvT[i],
            "wqT8": wqT8, "wkT8": wkT8, "wvT": wvT, "wpT": wpT,
            "bias": b2d,
        }
        for i in range(N)
    ]


def run(in_maps, trace=False, **kwargs):
    nc = _get_nc()
    return run_bass_kernel_spmd(nc, in_maps, list(range(N_CORES)), trace=trace, **kwargs)


def kernel(q, kv, wq, wk, wv, w_proj, b_proj):
    in_maps = make_in_maps(q, kv, wq, wk, wv, w_proj, b_proj)
    res = run(in_maps)
    return np.stack([res.results[i]["out"] for i in range(N_CORES)]).astype(np.float32)


# revision 7
# speedup vs baseline: 1.0063x; 1.0063x over previous
"""Cross-attention kernel v12 for Trainium2, data-parallel over batch across 8 cores.

v12 vs v7 (HW-microbenchmark-driven; v7 kept as kernel_v7_backup.py).
Fresh-process measurements sit in a ~133-150 us band (serialized-reps slope;
v7 baseline ~311-330 us). v12 = v10 plus a third score-PSUM slot (the exp
stream is the pacer; the extra slot decouples PE score matmuls from ACT's
exp reads), funded by shrinking the accumulator pool to 2 slots (safe since
the normalize evacuates PV output ~1 us after each chain).
Key changes vs v7:
  - PV matmul weights padded to 128 columns (V tile [P,SC,H,128]): LDWEIGHTS
    with NumWeights==128 triggers Fast Weight Load; 97-col weights measured
    677 ns/MM vs 179 ns/MM at 128 cols. Saves ~60 us of PE time.
  - Softmax normalize: PV output + denominator evacuated to SBUF right after
    each chain; the [1,512] single-lane reciprocal (3.2 us) replaced by a
    double-transpose batched reciprocal (32 lanes, ~0.3 us) in stock DVE ops
    (custom DVE ops are broken on HW through this PJRT path).
  - Phase-1 Q/K copies: ACT takes head 0 only so its FIFO reaches the first
    exp ~5 us in; DVE trickles the rest under early phase 2.
  - PE pre-warm matmuls during the DMA lead-in; one rearranged input DMA per
    tensor; one [128,768] f32 output DMA per q-block on the ACT HWDGE ring.
  - Output DMA: one [128,768] f32 transfer per q-block on the ACT HWDGE ring
    (SP ring is busy with input loads; ACT is idle in the tail).
"""

import sys

if "/opt/trn_rl_repo" not in sys.path:
    sys.path.insert(0, "/opt/trn_rl_repo")

import ml_dtypes
import numpy as np

import concourse.bass as bass
import concourse.mybir as mybir
from concourse import bacc
from concourse import library_config
import concourse.tile as tile
from concourse.bass_utils import run_bass_kernel_spmd

F32 = mybir.dt.float32
BF16 = mybir.dt.bfloat16
F8 = mybir.dt.float8e4

N, S, D = 8, 1024, 768
H, HD = 8, 96
P = 128
C = D // P
SC = S // P
W_SCALE = 16.0
SCALE = 1.0 / (float(np.sqrt(D)) * W_SCALE * W_SCALE)
N_CORES = 8
EST_BUFS = 24


def build_program(reps=1, serialize_reps=False):
    nc = bacc.Bacc(None, target_bir_lowering=False)

    qT8 = nc.dram_tensor("qT8", [D, S], F8, kind="ExternalInput")
    kvT8 = nc.dram_tensor("kvT8", [D, S], F8, kind="ExternalInput")
    wqT8 = nc.dram_tensor("wqT8", [D, D], F8, kind="ExternalInput")
    wkT8 = nc.dram_tensor("wkT8", [D, D], F8, kind="ExternalInput")
    kvT = nc.dram_tensor("kvT", [D, S], BF16, kind="ExternalInput")
    wvT = nc.dram_tensor("wvT", [D, D], BF16, kind="ExternalInput")
    wpT = nc.dram_tensor("wpT", [D, D], BF16, kind="ExternalInput")
    bias = nc.dram_tensor("bias", [1, D], F32, kind="ExternalInput")
    out = nc.dram_tensor("out", [S, D], F32, kind="ExternalOutput")

    with tile.TileContext(nc) as tc:
        persist = tc.alloc_tile_pool(name="persist", bufs=1)
        QT = persist.tile([HD, H, S], BF16, tag="QT")
        KT = persist.tile([HD, H, S], BF16, tag="KT")
        V = persist.tile([P, SC, H, P], BF16, tag="V")
        outhT = persist.tile([HD, H, S], BF16, tag="outhT")
        qa8 = persist.tile([P, C, S], F8, tag="qa8")
        kva8 = persist.tile([P, C, S], F8, tag="kva8")
        wq8 = persist.tile([P, C, D], F8, tag="wq8")
        wk8 = persist.tile([P, C, D], F8, tag="wk8")
        kva = persist.tile([P, C, S], BF16, tag="kva")
        wv_t = persist.tile([P, C, D], BF16, tag="wv")
        wp_t = persist.tile([HD, H, D], BF16, tag="wp")
        bias_t = persist.tile([1, D], F32, tag="bias")
        bias_bc = persist.tile([P, D], F32, tag="bias_bc")
        pwsrc = persist.tile([P, 512], BF16, tag="pwsrc")
        stgA = persist.tile([64, 512], BF16, tag="stgA")
        stgB = persist.tile([64, 512], BF16, tag="stgB")
        rstp = [
            [
                persist.tile([32, 512], BF16, tag=f"rst{a}{q}", name=f"rst{a}{q}")
                for q in range(2)
            ]
            for a in range(2)
        ]

        with (
            tc.tile_pool(name="expst", bufs=EST_BUFS) as estpool,
            tc.tile_pool(name="smmisc", bufs=2) as mpool,
            tc.tile_pool(name="pohp", bufs=4) as pohpool,
            tc.tile_pool(name="osb", bufs=3) as opool,
            tc.tile_pool(name="stpsum", bufs=3, space="PSUM") as stpsum,
            tc.tile_pool(name="accpsum", bufs=2, space="PSUM") as accpsum,
        ):
            for _rep in range(reps):
                if serialize_reps and _rep > 0:
                    tc.strict_bb_all_engine_barrier()
                nc.gpsimd.load_library(library_config.attn)
                # Pre-warm the Exp activation table during the DMA lead-in.
                warm = mpool.tile([1, 8], F32, tag="warm")
                warm_o = mpool.tile([1, 8], BF16, tag="warm_o")
                nc.vector.memset(warm[:], 0.0)
                nc.scalar.activation(
                    warm_o[:], warm[:], mybir.ActivationFunctionType.Exp, scale=1.0
                )
                if _rep == 0:
                    nc.vector.memset(pwsrc[:], 0.0)
                    nc.vector.memset(stgA[:], 1.0)
                    nc.vector.memset(stgB[:], 1.0)
                    for a in range(2):
                        for q in range(2):
                            nc.vector.memset(rstp[a][q][:], 1.0)
                # PE pre-warm: ~4us of dummy matmuls during the DMA lead-in so
                # the HAM clock-gate opens before the first real projection.
                for pw in range(8):
                    pwt = stpsum.tile([P, 2, 512], F32, tag="st", name=f"pw{pw}")
                    nc.tensor.matmul(
                        pwt[:, 0], pwsrc[:, 0:P], pwsrc[:], start=True, stop=True
                    )
                # Q/K fp8 path first: these gate the first matmuls. One
                # rearranged DMA each (the first projection chain needs the
                # whole tensor anyway; fewer DGE descriptor-gens frees the SP
                # ring sooner).
                nc.sync.dma_start(qa8[:], qT8.rearrange("(c p) s -> p c s", p=P))
                nc.sync.dma_start(wq8[:], wqT8.rearrange("(c p) o -> p c o", p=P))
                nc.sync.dma_start(kva8[:], kvT8.rearrange("(c p) s -> p c s", p=P))
                nc.sync.dma_start(wk8[:], wkT8.rearrange("(c p) o -> p c o", p=P))
                nc.sync.dma_start(kva[:], kvT.rearrange("(c p) s -> p c s", p=P))
                nc.sync.dma_start(wv_t[:], wvT.rearrange("(c p) o -> p c o", p=P))
                nc.sync.dma_start(wp_t[:], wpT.rearrange("(h d) o -> d h o", d=HD))
                nc.sync.dma_start(bias_t[:], bias[:, :])
                if _rep == 0:
                    # cols 96..127 of each V slice: ones (denominator row) then
                    # zeros so the junk PSUM rows 97..127 stay finite.
                    nc.vector.memset(V[:, :, :, HD:], 0.0)
                nc.vector.memset(V[:, :, :, HD], 1.0)
                nc.gpsimd.partition_broadcast(bias_bc[:], bias_t[:], channels=P)

                # ====== Phase 1: Q/K projections (fp8 DoubleRow) ======
                ci = 0
                for h in range(H):
                    for w8, acts, dstT in ((wq8, qa8, QT), (wk8, kva8, KT)):
                        for sh in range(2):
                            ps = accpsum.tile([P, 512], F32, tag="acc")
                            for cp in range(C // 2):
                                nc.tensor.matmul(
                                    ps[0:HD],
                                    w8[:, 2 * cp:2 * cp + 2, h * HD:(h + 1) * HD],
                                    acts[:, 2 * cp:2 * cp + 2, sh * 512:(sh + 1) * 512],
                                    start=(cp == 0),
                                    stop=(cp == C // 2 - 1),
                                    perf_mode=mybir.MatmulPerfMode.DoubleRow,
                                )
                            # ACT evacuates only head 0 (keeps its FIFO free
                            # so the exp stream starts ~5us in); DVE handles
                            # the rest, trickling into early phase 2.
                            if h == 0:
                                nc.scalar.copy(
                                    dstT[:, h, sh * 512:(sh + 1) * 512], ps[0:HD]
                                )
                            else:
                                nc.vector.tensor_copy(
                                    dstT[:, h, sh * 512:(sh + 1) * 512], ps[0:HD]
                                )
                            ci += 1

                # ====== Phase 2: slot-scheduled attention ======
                est_tiles = {}

                def emit_scores(h, kc):
                    st = stpsum.tile([P, 2, 512], F32, tag="st")
                    for qh in range(2):
                        nc.tensor.matmul(
                            st[:, qh],
                            KT[:, h, kc * P:(kc + 1) * P],
                            QT[:, h, qh * 512:(qh + 1) * 512],
                            start=True,
                            stop=True,
                        )
                    est = estpool.tile([P, 2, 512], BF16, tag="est")
                    nc.scalar.activation(
                        est[:], st[:], mybir.ActivationFunctionType.Exp, scale=SCALE
                    )
                    est_tiles[(h, kc)] = est

                # fp8 DoubleRow V projection: lhsT=kva8 chunk (the "weight"),
                # rhs=wv8 -> ps[s-block, dh*384..]; unscale 1/W_SCALE on copy.
                def emit_vproj(sc, dh):
                    psw = accpsum.tile([P, 512], F32, tag="acc", name=f"vp{sc}_{dh}")
                    ps = psw[:, 0:384]
                    for c in range(C):
                        nc.tensor.matmul(
                            ps[:],
                            kva[:, c, sc * P:(sc + 1) * P],
                            wv_t[:, c, dh * 384:(dh + 1) * 384],
                            start=(c == 0),
                            stop=(c == C - 1),
                        )
                    nc.vector.tensor_copy(
                        V[:, sc, dh * 4:(dh + 1) * 4, 0:HD],
                        ps[:].rearrange("p (h d) -> p h d", d=HD),
                    )

                # PV is emitted as per-slot matmul pairs; chain state per head.
                pv_state = {}

                def start_pv(h):
                    pv_state[h] = {"i": 0, "po": {}}

                def finish_norm(h, stt):
                    # double-transpose reciprocal: both qh denominator rows
                    # land strided over 32 partitions, reciprocal runs 32-wide
                    # (0.3us) instead of twice 3.2us single-lane.
                    stg = stt["stg"]
                    stgT = mpool.tile([64, 512], BF16, tag="stgT", name=f"sT{h}")
                    nc.vector.transpose(stgT[:], stg[:])
                    for qh in range(2):
                        # den row qh sits at stgT[32qh..32qh+31, 32j] after the
                        # block transpose; strided recip writes lane position 0
                        # so the back-transpose homes the row at partition 0
                        # (AP starts must be 0/32/64/96).
                        rst = rstp[h % 2][qh]
                        rin = stgT[32 * qh:32 * qh + 32].rearrange(
                            "p (j i) -> p j i", i=32
                        )[:, :, 0:1]
                        rout = rst[:].rearrange("p (j i) -> p j i", i=32)[
                            :, :, 0:1
                        ]
                        with nc.allow_low_precision(
                            reason="recip feeds bf16 outh scaling"
                        ):
                            nc.vector.reciprocal(rout, rin)
                        rT = mpool.tile(
                            [32, 512], BF16, tag=f"rT{qh}", name=f"rT{h}_{qh}"
                        )
                        nc.vector.transpose(rT[:], rst[:])
                        bc = mpool.tile(
                            [HD, 512], BF16, tag="bc", name=f"bc_{h}_{qh}"
                        )
                        nc.gpsimd.partition_broadcast(
                            bc[:], rT[0:1], channels=HD
                        )
                        nc.vector.tensor_mul(
                            outhT[:, h, qh * 512:(qh + 1) * 512],
                            stt[f"poh{qh}"][:], bc[:],
                        )

                def emit_pv_mms(h, count):
                    stt = pv_state[h]
                    for _ in range(count):
                        i = stt["i"]
                        if i >= 16:
                            return
                        qh, kc = divmod(i, SC)
                        if kc == 0:
                            stt["po"][qh] = accpsum.tile(
                                [P, 512], F32, tag="acc", name=f"po_{h}_{qh}"
                            )
                        po = stt["po"][qh]
                        nc.tensor.matmul(
                            po[:],
                            V[:, kc, h, :],
                            est_tiles[(h, kc)][:, qh],
                            start=(kc == 0),
                            stop=(kc == SC - 1),
                        )
                        stt["i"] = i + 1
                        if kc == SC - 1:
                            # evacuate po + denominator row, releasing the
                            # PSUM slot after two short copies
                            poh = pohpool.tile(
                                [HD, 512], BF16, tag="poh", name=f"poh_{h}_{qh}"
                            )
                            nc.vector.tensor_copy(poh[:], po[0:HD])
                            if qh == 0:
                                stt["stg"] = stgA if h % 2 == 0 else stgB
                            nc.vector.tensor_copy(
                                stt["stg"][32 * qh:32 * qh + 1], po[HD:HD + 1]
                            )
                            stt[f"poh{qh}"] = poh
                            if qh == 1:
                                finish_norm(h, stt)

                # V jobs dh-major: first 8 produce the V halves PV(h0..3) needs.
                vjobs = [(sc, dh) for dh in range(2) for sc in range(SC)]
                vj = 0
                for h in range(H):
                    for kc in range(SC):
                        emit_scores(h, kc)
                        if h < 4 and kc % 2 == 0 and vj < len(vjobs):
                            emit_vproj(*vjobs[vj])
                            vj += 1
                        if 2 <= h <= 5:
                            hh = h - 2
                            if hh not in pv_state:
                                start_pv(hh)
                            emit_pv_mms(hh, 2)
                        elif h == 6:
                            if kc < 4:
                                if 4 not in pv_state:
                                    start_pv(4)
                                emit_pv_mms(4, 4)
                            else:
                                if 5 not in pv_state:
                                    start_pv(5)
                                emit_pv_mms(5, 4)
                        elif h == 7:
                            if kc < 4:
                                if 6 not in pv_state:
                                    start_pv(6)
                                emit_pv_mms(6, 4)
                            if kc >= 1:
                                if 7 not in pv_state:
                                    start_pv(7)
                                emit_pv_mms(7, 1)
                    if 2 <= h <= 5:
                        emit_pv_mms(h - 2, 16)
                    elif h == 6:
                        emit_pv_mms(4, 16)
                        emit_pv_mms(5, 16)
                emit_pv_mms(6, 16)
                if 7 not in pv_state:
                    start_pv(7)
                emit_pv_mms(7, 16)

                # ====== Phase 3: output projection ======
                for qc in range(SC):
                    ot = opool.tile([P, D], F32, tag="ot")
                    for oh in range(2):
                        psw = accpsum.tile([P, 512], F32, tag="acc",
                                           name=f"op{qc}_{oh}")
                        ps = psw[:, 0:384]
                        for h in range(H):
                            nc.tensor.matmul(
                                ps[:],
                                outhT[:, h, qc * P:(qc + 1) * P],
                                wp_t[:, h, oh * 384:(oh + 1) * 384],
                                start=(h == 0),
                                stop=(h == H - 1),
                            )
                        nc.vector.tensor_add(
                            ot[:, oh * 384:(oh + 1) * 384], ps[:],
                            bias_bc[:, oh * 384:(oh + 1) * 384],
                        )
                    # One [128,768] f32 store per q-block on the ACT HWDGE
                    # ring (idle in the tail; SP ring carries the input loads).
                    nc.scalar.dma_start(out[qc * P:(qc + 1) * P, :], ot[:])

        persist.release()

    nc.compile()
    return nc


_NC_CACHE = {}


def _get_nc(reps=1, serialize_reps=False):
    key = (reps, serialize_reps)
    if key not in _NC_CACHE:
        _NC_CACHE[key] = build_program(reps, serialize_reps)
    return _NC_CACHE[key]


def _bf16(x):
    return np.ascontiguousarray(np.asarray(x, np.float32).astype(ml_dtypes.bfloat16))


def _f8(x):
    return np.ascontiguousarray(np.asarray(x, np.float32).astype(ml_dtypes.float8_e4m3))


def make_in_maps(q, kv, wq, wk, wv, w_proj, b_proj):
    q = np.asarray(q, np.float32)
    kv = np.asarray(kv, np.float32)
    qT8 = _f8(q.transpose(0, 2, 1))
    kvT8 = _f8(kv.transpose(0, 2, 1))
    wqT8 = _f8(np.asarray(wq, np.float32).T * W_SCALE)
    wkT8 = _f8(np.asarray(wk, np.float32).T * W_SCALE)
    kvT = _bf16(kv.transpose(0, 2, 1))
    wvT = _bf16(np.asarray(wv, np.float32).T)
    wpT = _bf16(np.asarray(w_proj, np.float32).T)
    b2d = np.ascontiguousarray(np.asarray(b_proj, np.float32).reshape(1, D))
    return [
        {
            "qT8": qT8[i], "kvT8": kvT8[i], "kvT": kvT[i],
            "wqT8": wqT8, "wkT8": wkT8, "wvT": wvT, "wpT": wpT,
            "bias": b2d,
        }
        for i in range(N)
    ]


def run(in_maps, trace=False, **kwargs):
    nc = _get_nc()
    return run_bass_kernel_spmd(nc, in_maps, list(range(N_CORES)), trace=trace, **kwargs)


def kernel(q, kv, wq, wk, wv, w_proj, b_proj):
    in_maps = make_in_maps(q, kv, wq, wk, wv, w_proj, b_proj)
    res = run(in_maps)
    return np.stack([res.results[i]["out"] for i in range(N_CORES)]).astype(np.float32)
